# revision 11
# baseline (speedup 1.0000x reference)
"""DispersionLoss kernel for Trainium2 (8 NeuronCores, Bass/Tile).

Reference computation (N=16384, F=64, K=32, C=128):
    bin_mass[f,k]  = sum_n m[n,f,k] + EPS
    SWY[f,k,c]     = sum_n m[n,f,k] * y[n,c]
    cent[f,k,c]    = SWY / bin_mass
    loss_dispersion= sum_fk ( sum_n m*dist2 ) / bin_mass
                   = sum_fk ( A/bin_mass - c_sq - EPS*c_sq/bin_mass )
        where A[f,k] = sum_n m[n,f,k]*|y_n|^2   (algebraic expansion: the
        cross term sum_n m*cross equals bin_mass*c_sq exactly)
    loss_entropy   = sum_fk p*log(p+EPS), p = bin_mass/N
    loss_repulsion = sum_f sum_k exp(-|cent[f,k]-cent[f,k+1]|^2)
    loss_inter     = sum_f sum_{k<j} exp(-|cent[f,k]-cent[f,j]|^2) / F
                   = sum_f (sum_{kj} exp(-pairwise) - K) / 2 / F   (symmetry)

Sharding: over F (8 features per core) -> every loss term decomposes per-f,
so no cross-core collectives are needed; host sums 8 partial scalars.

Each core:
  phase 1: one pass over its (16384 x 256) membership slice.  Per 128-row
    tile: matmul psum[fk=128, 130] += G_half.T @ [Y | y_sq | 1]   (x2 halves)
  phase 2: tiny (256 bins) per-bin math + per-f (32x32) pairwise stage,
    emitting 5 partial scalars.
"""

import os
import numpy as np

N = 16384
F = 64
K = 32
C = 128
NCORES = 8
F_PER_CORE = F // NCORES          # 8
FK = F_PER_CORE * K               # 256 bins per core
NT = N // 128                     # 128 row-tiles

LAMBDA_ENTROPY = 0.1
LAMBDA_REPULSION = 0.5
LAMBDA_INTER = 0.3
EPS = 1e-8

_NC_CACHE = {}


def _build_nc(use_f32r: bool):
    import concourse.bacc as bacc
    import concourse.tile as tile
    from concourse import mybir

    f32 = mybir.dt.float32

    nc = bacc.Bacc("TRN2", target_bir_lowering=False, debug=False)
    g_dram = nc.dram_tensor("g", (N, FK), f32, kind="ExternalInput").ap()
    y_dram = nc.dram_tensor("y", (N, C), f32, kind="ExternalInput").ap()
    out_dram = nc.dram_tensor("out", (1, 8), f32, kind="ExternalOutput").ap()

    with tile.TileContext(nc) as tc:
        with (
            tc.tile_pool(name="singles", bufs=1) as singles,
            tc.tile_pool(name="gpool", bufs=4) as gpool,
            tc.tile_pool(name="ypool", bufs=4) as ypool,
            tc.tile_pool(name="scr", bufs=2) as scr,
            tc.tile_pool(name="ph2", bufs=1) as ph2,
            tc.tile_pool(name="psacc", bufs=1, space="PSUM") as psacc,
            tc.tile_pool(name="pstmp", bufs=2, space="PSUM") as pstmp,
        ):
            # ---- constants ----
            mi2 = singles.tile([128, 128], f32)          # -2 * identity
            nc.gpsimd.memset(mi2, 0.0)
            nc.gpsimd.affine_select(
                out=mi2, in_=mi2,
                compare_op=mybir.AluOpType.not_equal,
                fill=-2.0, base=0, pattern=[[-1, 128]], channel_multiplier=1,
            )
            ones128 = singles.tile([128, 1], f32)
            nc.gpsimd.memset(ones128, 1.0)
            quarter = singles.tile([128, 1], f32)
            nc.gpsimd.memset(quarter, 0.25)
            eps128 = singles.tile([128, 1], f32)
            nc.gpsimd.memset(eps128, EPS)
            interb = singles.tile([1, 1], f32)
            nc.gpsimd.memset(interb, -(F_PER_CORE * K) / (2.0 * F))

            # ---- phase 1: G^T @ [Y | y_sq | 1] accumulated over row tiles ----
            ps = [psacc.tile([128, 130], f32, tag=f"ps{h}", name=f"ps{h}") for h in range(2)]
            for ti in range(NT):
                r0 = ti * 128
                g = gpool.tile([128, FK], f32)
                nc.sync.dma_start(out=g, in_=g_dram[r0:r0 + 128, :])
                yx = ypool.tile([128, 130], f32)
                nc.sync.dma_start(out=yx[:, 0:C], in_=y_dram[r0:r0 + 128, :])
                sq = scr.tile([128, C], f32)
                nc.scalar.activation(
                    out=sq, in_=yx[:, 0:C],
                    func=mybir.ActivationFunctionType.Square,
                    accum_out=yx[:, C:C + 1],
                )
                nc.gpsimd.memset(yx[:, C + 1:C + 2], 1.0)
                for h in range(2):
                    lhsT = g[:, h * 128:(h + 1) * 128]
                    rhs = yx[:, :]
                    if use_f32r:
                        lhsT = lhsT.bitcast(mybir.dt.float32r)
                        rhs = rhs.bitcast(mybir.dt.float32r)
                    nc.tensor.matmul(
                        ps[h], lhsT, rhs,
                        start=(ti == 0), stop=(ti == NT - 1),
                    )

            # ---- phase 2 ----
            m2 = ph2.tile([128, FK], f32)        # -2 * cent^T   (c x fk)
            top = ph2.tile([2, FK], f32)         # [1 ; c_sq]
            bot = ph2.tile([2, FK], f32)         # [-2*c_sq ; -2]
            st_parts = []
            for h in range(2):
                p = ps[h]
                cs = h * 128
                bin_mass = ph2.tile([128, 1], f32, tag=f"bm{h}")
                nc.scalar.activation(
                    out=bin_mass, in_=p[:, 129:130],
                    func=mybir.ActivationFunctionType.Identity,
                    bias=eps128, scale=1.0,
                )
                inv = ph2.tile([128, 1], f32, tag=f"inv{h}")
                nc.vector.reciprocal(inv, bin_mass)
                cent = ph2.tile([128, 128], f32, tag=f"cent{h}")
                nc.vector.tensor_scalar_mul(cent, in0=p[:, 0:C], scalar1=inv)
                csq_scr = scr.tile([128, 128], f32, tag="csqscr")
                c_sq = ph2.tile([128, 1], f32, tag=f"csq{h}")
                nc.scalar.activation(
                    out=csq_scr, in_=cent,
                    func=mybir.ActivationFunctionType.Square,
                    accum_out=c_sq,
                )
                # wv = A*inv - c_sq - EPS*c_sq*inv ; ent = p*ln(p+EPS)
                st = ph2.tile([128, 2], f32, tag=f"st{h}")
                t0 = ph2.tile([128, 1], f32, tag=f"t0{h}")
                nc.vector.tensor_mul(t0, p[:, 128:129], inv)
                nc.vector.tensor_sub(t0, t0, c_sq)
                t1 = ph2.tile([128, 1], f32, tag=f"t1{h}")
                nc.vector.tensor_mul(t1, c_sq, inv)
                nc.scalar.mul(t1, t1, EPS)
                nc.vector.tensor_sub(st[:, 0:1], t0, t1)
                pp = ph2.tile([128, 1], f32, tag=f"pp{h}")
                nc.scalar.mul(pp, bin_mass, 1.0 / N)
                lg = ph2.tile([128, 1], f32, tag=f"lg{h}")
                nc.scalar.activation(
                    out=lg, in_=pp,
                    func=mybir.ActivationFunctionType.Ln,
                    bias=eps128, scale=1.0,
                )
                nc.vector.tensor_mul(st[:, 1:2], pp, lg)
                # partition-reduce [wv, ent] via ones-matmul
                ps_st = pstmp.tile([1, 2], f32, tag="pstmp", name=f"ps_st{h}")
                nc.tensor.matmul(ps_st, ones128, st, start=True, stop=True)
                st_sb = ph2.tile([1, 2], f32, tag=f"stsb{h}")
                nc.scalar.copy(st_sb, ps_st)
                st_parts.append(st_sb)
                # m2 half: cent^T @ (-2 I)
                ps_m2 = pstmp.tile([128, 128], f32, tag="pstmp")
                nc.tensor.matmul(ps_m2, cent, mi2, start=True, stop=True)
                nc.scalar.copy(m2[:, cs:cs + 128], ps_m2)
                # top rows [1 ; c_sq] = -0.5 * ([ones|c_sq]^T @ -2I)
                # bot rows [-2*c_sq ; -2] = [c_sq|ones]^T @ -2I
                sc_t = ph2.tile([128, 2], f32, tag=f"sct{h}")
                nc.vector.tensor_copy(sc_t[:, 0:1], ones128)
                nc.vector.tensor_copy(sc_t[:, 1:2], c_sq)
                sc_b = ph2.tile([128, 2], f32, tag=f"scb{h}")
                nc.vector.tensor_copy(sc_b[:, 0:1], c_sq)
                nc.vector.tensor_copy(sc_b[:, 1:2], ones128)
                ps_t2 = pstmp.tile([2, 128], f32, tag="pstmp", name=f"ps_t2{h}")
                nc.tensor.matmul(ps_t2, sc_t, mi2, start=True, stop=True)
                nc.scalar.mul(top[0:2, cs:cs + 128], ps_t2, -0.5)
                ps_b2 = pstmp.tile([2, 128], f32, tag="pstmp", name=f"ps_b2{h}")
                nc.tensor.matmul(ps_b2, sc_b, mi2, start=True, stop=True)
                nc.scalar.copy(bot[0:2, cs:cs + 128], ps_b2)

            stats = ph2.tile([1, 2], f32)
            nc.vector.tensor_add(stats, st_parts[0], st_parts[1])

            # ---- repulsion: adjacent-bin distances from m2 = -2 cent^T ----
            dd = ph2.tile([128, FK - 1], f32)
            nc.vector.tensor_sub(dd, m2[:, 0:FK - 1], m2[:, 1:FK])
            nc.scalar.activation(
                out=dd, in_=dd, func=mybir.ActivationFunctionType.Square,
            )
            ps_nd = pstmp.tile([1, FK - 1], f32, tag="pstmp")
            nc.tensor.matmul(ps_nd, quarter, dd, start=True, stop=True)
            en = ph2.tile([1, FK - 1], f32)
            en_tot = ph2.tile([1, 1], f32)
            nc.scalar.activation(
                out=en, in_=ps_nd, func=mybir.ActivationFunctionType.Exp,
                scale=-1.0, accum_out=en_tot,
            )
            # subtract the K-1 -> K boundary columns (k==31 of f=0..6)
            inv_view = en[0:1, 0:(F_PER_CORE - 1) * K].rearrange(
                "p (a b) -> p a b", b=K
            )[:, :, K - 1:K]
            inv_sum = ph2.tile([1, 1], f32)
            nc.vector.reduce_sum(inv_sum, inv_view, axis=mybir.AxisListType.XY)
            repl = ph2.tile([1, 1], f32)
            nc.vector.tensor_sub(repl, en_tot, inv_sum)

            # ---- inter: per-f all-pairs E = exp(-pairwise) ----
            erows = ph2.tile([K, F_PER_CORE], f32)
            for f in range(F_PER_CORE):
                fs = f * K
                pw = pstmp.tile([K, K], f32, tag="pw")
                nc.tensor.matmul(pw, m2[:, fs:fs + K], m2[:, fs:fs + K],
                                 start=True, stop=False)
                nc.tensor.matmul(pw, top[:, fs:fs + K], bot[:, fs:fs + K],
                                 start=False, stop=True)
                e_scr = scr.tile([K, K], f32, tag="escr")
                nc.scalar.activation(
                    out=e_scr, in_=pw, func=mybir.ActivationFunctionType.Exp,
                    scale=0.5, accum_out=erows[:, f:f + 1],
                )
            rowtot = ph2.tile([K, 1], f32)
            nc.vector.reduce_sum(rowtot, erows, axis=mybir.AxisListType.X)
            ps_i = pstmp.tile([1, 1], f32, tag="pstmp")
            nc.tensor.matmul(ps_i, ones128[0:K, :], rowtot, start=True, stop=True)
            inter = ph2.tile([1, 1], f32)
            # (allsum - F_PER_CORE*K) / 2 / F
            nc.scalar.activation(
                out=inter, in_=ps_i,
                func=mybir.ActivationFunctionType.Identity,
                scale=1.0 / (2 * F), bias=interb,
            )

            # ---- final: [total, disp, ent, repl, inter] ----
            res = ph2.tile([1, 8], f32)
            nc.gpsimd.memset(res, 0.0)
            nc.scalar.copy(res[0:1, 1:2], stats[0:1, 0:1])
            nc.scalar.copy(res[0:1, 2:3], stats[0:1, 1:2])
            nc.scalar.copy(res[0:1, 3:4], repl)
            nc.scalar.copy(res[0:1, 4:5], inter)
            tot = ph2.tile([1, 1], f32)
            tmp = ph2.tile([1, 1], f32)
            nc.scalar.mul(tmp, stats[0:1, 1:2], LAMBDA_ENTROPY)
            nc.vector.tensor_add(tot, stats[0:1, 0:1], tmp)
            nc.scalar.mul(tmp, repl, LAMBDA_REPULSION)
            nc.vector.tensor_add(tot, tot, tmp)
            nc.scalar.mul(tmp, inter, LAMBDA_INTER)
            nc.vector.tensor_add(tot, tot, tmp)
            nc.scalar.copy(res[0:1, 0:1], tot)
            nc.sync.dma_start(out=out_dram, in_=res)

    nc.compile()
    return nc


def get_nc(use_f32r: bool = True):
    key = bool(use_f32r)
    if key not in _NC_CACHE:
        _NC_CACHE[key] = _build_nc(key)
    return _NC_CACHE[key]


def kernel(membership: np.ndarray, teacher_preds: np.ndarray, _trace: bool = False,
           _use_f32r: bool = True):
    from concourse.bass_utils import run_bass_kernel_spmd

    m = np.ascontiguousarray(np.asarray(membership, dtype=np.float32)).reshape(N, F * K)
    y = np.ascontiguousarray(np.asarray(teacher_preds, dtype=np.float32))

    nc = get_nc(_use_f32r)
    in_maps = []
    for i in range(NCORES):
        in_maps.append({
            "g": np.ascontiguousarray(m[:, i * FK:(i + 1) * FK]),
            "y": y,
        })
    res = run_bass_kernel_spmd(
        nc, in_maps, core_ids=list(range(NCORES)), trace=_trace,
    )
    parts = np.stack(
        [np.asarray(res.results[i]["out"][0], dtype=np.float64) for i in range(NCORES)]
    )
    tot = parts.sum(axis=0)
    out = tuple(np.float32(tot[j]) for j in range(5))
    if _trace:
        return out, res
    return out


if __name__ == "__main__":
    rng = np.random.default_rng(0)
    mem = rng.random((N, F, K), dtype=np.float32)
    tp = rng.random((N, C), dtype=np.float32)
    print(kernel(mem, tp))
